# revision 17
# baseline (speedup 1.0000x reference)
"""Trainium2 Bass kernel for nn_MeanAggregator (GNN mean aggregation).

out[b] = relu(concat(features[node[b]], mean_k features[neighbours[b,k]]) @ W)

8 NeuronCores, data-parallel over the batch (4096 items/core).  Tolerance is
2e-2, so features/W are cast to bf16 on the host (measured end-to-end error
~2.5e-3).

Gather strategy: `indirect_dma_start` costs ~1.1 us of Q7/SWDGE time per call
and can only gather 128 rows (one index per partition), so the 26 rows/item
gather would be Q7-bound.  Instead we use the vectorized `dma_gather`
(transpose=True): one instruction gathers 26624 rows (26 slots x 1024 items)
with ~0.34 ns/descriptor of Q7 time and lands them TRANSPOSED in SBUF as
[dim, slot-major columns] via the XBAR spray path.

dma_gather indices are int16 (15-bit usable), so the host builds, per
quarter-core (1024 items), a deduplicated sub-table of the ~23.4k unique
referenced rows (< 32767) and remaps indices into it.  The device then does
the full 26-rows/item expansion from HBM.

In transposed layout the whole aggregation collapses into matmuls: for each
512-item group, psum[u, b] += sum_d Wc[d, u] * gT[d, (c, b)] accumulated over
the 26 slots c (Wc = W_top for the node slot, W_bot/25 for neighbour slots)
— no PE transposes, no DVE tree, exact f32 PSUM accumulation.  ACT applies
relu; the output is produced transposed [units, items] and the host
transposes it back.
"""

import sys

sys.path.insert(0, "/opt/trn_rl_repo")

import numpy as np

from concourse import bacc, bass, mybir, tile
from concourse.bass_utils import run_bass_kernel_spmd

N_NODES = 100000
DIM = 128
B = 32768
K = 25
UNITS = 128
N_CORES = 8
P = 128
IDX_W = K + 1

Q_ITEMS = 1024  # items per gather batch (quarter of a core)
G_ITEMS = 512  # items per matmul group (one PSUM bank of f32)
N_GRP = Q_ITEMS // G_ITEMS
NI = IDX_W * Q_ITEMS  # indices per gather batch
IDX_COLS = NI // 16
U_MAX = 25000  # sub-table capacity (unique rows per batch ~23.4k +- 0.1k)

# one transposing dma_gather caps at 896 indices on real HW (1024 x 256 B
# hits a 2^18-byte limit in the descriptor path; HW-probed 896 ok / 1024
# wedges the device); chunk each group's gather accordingly
NI_G = IDX_W * G_ITEMS  # indices per matmul group
NI_CHUNK = 896
_bounds = list(range(0, NI_G, NI_CHUNK))
CHUNKS = [(a, min(a + NI_CHUNK, NI_G)) for a in _bounds]

BF16_NP = mybir.dt.np(mybir.dt.bfloat16)


def build_program(n_items):
    n_q = n_items // Q_ITEMS
    assert n_items % Q_ITEMS == 0

    nc = bacc.Bacc("TRN2", target_bir_lowering=False, debug=False,
                   num_swdge_queues=4)
    f32 = mybir.dt.float32
    bf16 = mybir.dt.bfloat16
    subtab = nc.dram_tensor(
        "subtab", [n_q, U_MAX, DIM], bf16, kind="ExternalInput"
    ).ap()
    idx16 = nc.dram_tensor(
        "idx16", [n_q, P, IDX_COLS], mybir.dt.int16, kind="ExternalInput"
    ).ap()
    wt = nc.dram_tensor("wt", [DIM, UNITS], bf16, kind="ExternalInput").ap()
    wb = nc.dram_tensor("wb", [DIM, UNITS], bf16, kind="ExternalInput").ap()
    outT = nc.dram_tensor("outT", [UNITS, n_items], f32, kind="ExternalOutput").ap()

    relu = mybir.ActivationFunctionType.Relu

    with tile.TileContext(nc) as tc:
        with (
            tc.tile_pool(name="const", bufs=1) as cpool,
            tc.tile_pool(name="gpool", bufs=4) as gpool,
            tc.tile_pool(name="opool", bufs=3) as opool,
            tc.tile_pool(name="pp", bufs=2, space="PSUM") as pp,
            tc.tile_pool(name="ppw", bufs=1, space="PSUM") as ppw,
        ):
            wt_sb = cpool.tile([DIM, UNITS], bf16, tag="wt")
            nc.sync.dma_start(out=wt_sb[:], in_=wt[:])
            wb_sb = cpool.tile([DIM, UNITS], bf16, tag="wb")
            nc.sync.dma_start(out=wb_sb[:], in_=wb[:])

            # idx_sb[p, q*IDX_COLS + c] = idx16[q, p, c]
            idx_sb = cpool.tile([P, n_q * IDX_COLS], mybir.dt.int16, tag="idx")
            nc.sync.dma_start(
                out=idx_sb[:].rearrange("p (q c) -> p q c", c=IDX_COLS),
                in_=idx16.rearrange("q p c -> p q c"),
            )

            # warmup matmuls absorb the constant-load waits
            psum_warm = ppw.tile([P, UNITS], f32, tag="warm")
            nc.tensor.matmul(out=psum_warm[:], lhsT=wt_sb[:], rhs=wt_sb[:])
            nc.tensor.matmul(out=psum_warm[:], lhsT=wb_sb[:], rhs=wb_sb[:])

            chunk_no = 0
            for q in range(n_q):
                for g in range(N_GRP):
                    grp = q * N_GRP + g
                    # transposing gather of 512 items x 26 slots, chunked to
                    # fit the SWDGE ring: gT[d, c*G_ITEMS + b] =
                    # subtab[q, idx[(c,b)], d].  queue_num must track the
                    # tile scheduler's DMASW lane rotation (lane = Pool-DMA
                    # counter % 8, queue = counter % 4) or a sem lane gets
                    # updates from two queues (CoreSim errors; HW corrupts).
                    gT = gpool.tile([P, NI_G], bf16, tag="gT")
                    goff = q * IDX_COLS + g * (NI_G // 16)
                    for a, b in CHUNKS:
                        n = b - a
                        nc.gpsimd.dma_gather(
                            gT[:, a:b].rearrange("p (x n) -> p x n", x=1),
                            subtab[q],
                            idx_sb[:, goff + a // 16 : goff + b // 16],
                            n,
                            n,
                            DIM,
                            transpose=True,
                            queue_num=grp % 4,
                        )
                        chunk_no += 1

                    psum = pp.tile([P, G_ITEMS], f32, tag="ps")
                    for c in range(IDX_W):
                        col = c * G_ITEMS
                        nc.tensor.matmul(
                            out=psum[:],
                            lhsT=(wt_sb if c == 0 else wb_sb)[:],
                            rhs=gT[:, col : col + G_ITEMS],
                            start=(c == 0),
                            stop=(c == IDX_W - 1),
                        )
                    o_sb = opool.tile([P, G_ITEMS], f32, tag="osb")
                    nc.scalar.activation(out=o_sb[:], in_=psum[:], func=relu)
                    col = grp * G_ITEMS
                    nc.sync.dma_start(
                        out=outT[:, col : col + G_ITEMS], in_=o_sb[:]
                    )

    nc.compile()
    return nc


_PROGRAM_CACHE = {}


def _get_program(n_items):
    if n_items not in _PROGRAM_CACHE:
        _PROGRAM_CACHE[n_items] = build_program(n_items)
    return _PROGRAM_CACHE[n_items]


def _prep_core(features_bf, idx_core):
    """Per-core host prep: dedup per quarter, build sub-tables + wrapped
    int16 slot-major indices."""
    n_q = idx_core.shape[0] // Q_ITEMS
    subtab = np.zeros((n_q, U_MAX, DIM), dtype=BF16_NP)
    idx16 = np.empty((n_q, P, IDX_COLS), dtype=np.int16)
    for q in range(n_q):
        sl = idx_core[q * Q_ITEMS : (q + 1) * Q_ITEMS]  # [Q_ITEMS, IDX_W]
        uniq, inv = np.unique(sl, return_inverse=True)
        assert len(uniq) <= U_MAX, f"unique rows {len(uniq)} > {U_MAX}"
        subtab[q, : len(uniq)] = features_bf[uniq]
        inv = inv.reshape(Q_ITEMS, IDX_W).astype(np.int16)
        # index order j = (g*IDX_W + c)*G_ITEMS + b
        ordered = np.empty(NI, dtype=np.int16)
        for g in range(N_GRP):
            blk = inv[g * G_ITEMS : (g + 1) * G_ITEMS]  # [G_ITEMS, IDX_W]
            ordered[g * IDX_W * G_ITEMS : (g + 1) * IDX_W * G_ITEMS] = (
                blk.T.ravel()
            )
        # wrap each gather chunk into 16 partitions (within-chunk
        # j = col*16 + p), replicate to 128
        wrapped = np.concatenate(
            [
                ordered[g * NI_G + a : g * NI_G + b].reshape(-1, 16).T
                for g in range(N_GRP)
                for a, b in CHUNKS
            ],
            axis=1,
        )
        idx16[q] = np.tile(wrapped, (8, 1))
    return subtab, idx16


def _prep_inputs(features, node, neighbours, W):
    features_bf = np.asarray(features, dtype=np.float32).astype(BF16_NP)
    node = np.asarray(node, dtype=np.int32).reshape(-1, 1)
    neighbours = np.asarray(neighbours, dtype=np.int32)
    W = np.asarray(W, dtype=np.float32)
    idx_all = np.ascontiguousarray(
        np.concatenate([node, neighbours], axis=1), dtype=np.int32
    )
    wt = np.ascontiguousarray(W[:DIM]).astype(BF16_NP)
    wb = (W[DIM:].astype(np.float64) / K).astype(BF16_NP)
    return features_bf, idx_all, wt, wb


def kernel(features, node, neighbours, W, trace=False):
    features_bf, idx_all, wt, wb = _prep_inputs(features, node, neighbours, W)
    n_total = idx_all.shape[0]
    per_core = n_total // N_CORES
    nc = _get_program(per_core)
    in_maps = []
    for i in range(N_CORES):
        subtab, idx16 = _prep_core(
            features_bf, idx_all[i * per_core : (i + 1) * per_core]
        )
        in_maps.append({"subtab": subtab, "idx16": idx16, "wt": wt, "wb": wb})
    res = run_bass_kernel_spmd(nc, in_maps, list(range(N_CORES)), trace=trace)
    out = np.ascontiguousarray(
        np.concatenate([res.results[i]["outT"] for i in range(N_CORES)], axis=1).T
    )
    if trace:
        kernel.last_result = res
    return out


# revision 18
# speedup vs baseline: 2.6813x; 2.6813x over previous
"""Trainium2 Bass kernel for nn_MeanAggregator (GNN mean aggregation).

out[b] = relu(concat(features[node[b]], mean_k features[neighbours[b,k]]) @ W)

8 NeuronCores, data-parallel over the batch (4096 items/core).  Tolerance is
2e-2, so features/W are cast to bf16 on the host (measured end-to-end error
~2.5e-3).

Gather strategy: the only SWDGE path whose Q7 descriptor generation is
vectorized is `dma_gather` (~3.5 ns/row vs ~1.1 us per 128-row
`indirect_dma_start`).  Its indices are int16 (15-bit usable), so the host
builds, per quarter-core (1024 items), a deduplicated sub-table of the
~23.4k unique referenced rows (< 32767) and remaps indices into it; the
device does the full 26-rows/item expansion from HBM.  One dma_gather is
capped at 1024 indices (256 B rows; the descriptor path tops out at 2^18
bytes — HW-probed), so each quarter issues 26 chunks of 1024 rows, spread
round-robin over 4 SWDGE queues so all four Q7 core-pairs generate
descriptors in parallel.  transpose=True (XBAR spray) is NOT used: under
multi-queue load its rx/tx rings desync and chunk k+1's data lands on chunk
k's columns (HW-probed, deterministic).  Plain gathers put row j in
partition j%128, block j//128, so ordering indices as j = (tile*26 + slot)
*128 + p reproduces the [item-partition, slot-block] layout directly.

Compute per 128-item tile: DVE pairwise tree (bf16, 2 elem/cyc) sums the 25
neighbour slots; PE transposes node + neighbour-sum into [dim, item] (2
transpose matmuls) and applies W_top / W_bot/25 with PSUM accumulation; ACT
does the PSUM->SBUF copy and the relu so DVE/PE stay on their critical
paths.
"""

import sys

sys.path.insert(0, "/opt/trn_rl_repo")

import numpy as np

from concourse import bacc, bass, mybir, tile
from concourse.bass_utils import run_bass_kernel_spmd
from concourse.masks import make_identity

N_NODES = 100000
DIM = 128
B = 32768
K = 25
UNITS = 128
N_CORES = 8
P = 128
IDX_W = K + 1

Q_ITEMS = 1024  # items per gather batch (quarter of a core)
T_ITEMS = 128  # items per compute tile
N_TILES = Q_ITEMS // T_ITEMS
NI = IDX_W * Q_ITEMS  # indices per gather batch (26624)
TI = IDX_W * T_ITEMS  # indices per tile (3328)
IDX_COLS = NI // 16
U_MAX = 25000  # sub-table capacity (unique rows per batch ~23.4k +- 0.1k)
NI_CHUNK = 1024  # max indices per dma_gather (non-transpose, HW-probed)
N_CHUNK = NI // NI_CHUNK  # 26 chunks per quarter

BF16_NP = mybir.dt.np(mybir.dt.bfloat16)


def build_program(n_items):
    n_q = n_items // Q_ITEMS
    assert n_items % Q_ITEMS == 0

    nc = bacc.Bacc("TRN2", target_bir_lowering=False, debug=False,
                   num_swdge_queues=4)
    f32 = mybir.dt.float32
    bf16 = mybir.dt.bfloat16
    subtab = nc.dram_tensor(
        "subtab", [n_q, U_MAX, DIM], bf16, kind="ExternalInput"
    ).ap()
    idx16 = nc.dram_tensor(
        "idx16", [n_q, P, IDX_COLS], mybir.dt.int16, kind="ExternalInput"
    ).ap()
    wt = nc.dram_tensor("wt", [DIM, UNITS], bf16, kind="ExternalInput").ap()
    wb = nc.dram_tensor("wb", [DIM, UNITS], bf16, kind="ExternalInput").ap()
    out = nc.dram_tensor("out", [n_items, UNITS], f32, kind="ExternalOutput").ap()

    relu = mybir.ActivationFunctionType.Relu
    copyf = mybir.ActivationFunctionType.Copy

    with tile.TileContext(nc) as tc:
        with (
            tc.tile_pool(name="const", bufs=1) as cpool,
            tc.tile_pool(name="gpool", bufs=2) as gpool,
            tc.tile_pool(name="spool", bufs=2) as spool,
            tc.tile_pool(name="opool", bufs=3) as opool,
            tc.tile_pool(name="pp", bufs=2, space="PSUM") as pp,
            tc.tile_pool(name="ppw", bufs=1, space="PSUM") as ppw,
        ):
            wt_sb = cpool.tile([DIM, UNITS], bf16, tag="wt")
            nc.sync.dma_start(out=wt_sb[:], in_=wt[:])
            wb_sb = cpool.tile([DIM, UNITS], bf16, tag="wb")
            nc.sync.dma_start(out=wb_sb[:], in_=wb[:])
            ident = cpool.tile([P, P], bf16, tag="ident")
            make_identity(nc, ident[:])

            # idx_sb[p, q*IDX_COLS + c] = idx16[q, p, c]
            idx_sb = cpool.tile([P, n_q * IDX_COLS], mybir.dt.int16, tag="idx")
            nc.sync.dma_start(
                out=idx_sb[:].rearrange("p (q c) -> p q c", c=IDX_COLS),
                in_=idx16.rearrange("q p c -> p q c"),
            )

            # warmup matmuls absorb the constant-load waits
            psum_warm_t = ppw.tile([P, UNITS], bf16, tag="warmt")
            nc.tensor.matmul(
                out=psum_warm_t[:], lhsT=ident[:], rhs=ident[:], is_transpose=True
            )
            psum_warm = ppw.tile([P, UNITS], f32, tag="warm")
            nc.tensor.matmul(out=psum_warm[:], lhsT=wt_sb[:], rhs=wt_sb[:])
            nc.tensor.matmul(out=psum_warm[:], lhsT=wb_sb[:], rhs=wb_sb[:])

            chunk_no = 0
            for q in range(n_q):
                # plain gather of 26624 rows: row j -> partition j%128,
                # block j//128; j = (t*26 + c)*128 + p, so tile t's slot c
                # sits at gq[:, (t*26+c)*128 : +128]
                gq = gpool.tile([P, NI], bf16, tag="gq")
                for k in range(N_CHUNK):
                    a = k * NI_CHUNK
                    nc.gpsimd.dma_gather(
                        gq[:, a : a + NI_CHUNK].rearrange(
                            "p (n e) -> p n e", e=DIM
                        ),
                        subtab[q],
                        idx_sb[
                            :,
                            q * IDX_COLS + a // 16 : q * IDX_COLS
                            + (a + NI_CHUNK) // 16,
                        ],
                        NI_CHUNK,
                        NI_CHUNK,
                        DIM,
                        transpose=False,
                        queue_num=chunk_no % 4,
                    )
                    chunk_no += 1

                for t in range(N_TILES):
                    base = t * TI  # element column offset of tile t
                    # DVE pairwise tree over neighbour slots 1..25
                    s12 = spool.tile([P, 12 * DIM], bf16, tag="s12")
                    nc.vector.tensor_add(
                        s12[:],
                        gq[:, base + 1 * DIM : base + 13 * DIM],
                        gq[:, base + 13 * DIM : base + 25 * DIM],
                    )
                    s6 = spool.tile([P, 6 * DIM], bf16, tag="s6")
                    nc.vector.tensor_add(
                        s6[:], s12[:, : 6 * DIM], s12[:, 6 * DIM :]
                    )
                    s3 = spool.tile([P, 3 * DIM], bf16, tag="s3")
                    nc.vector.tensor_add(
                        s3[:], s6[:, : 3 * DIM], s6[:, 3 * DIM :]
                    )
                    p1 = spool.tile([P, DIM], bf16, tag="p1")
                    nc.vector.tensor_add(
                        p1[:], s3[:, :DIM], s3[:, DIM : 2 * DIM]
                    )
                    p2 = spool.tile([P, DIM], bf16, tag="p2")
                    nc.vector.tensor_add(p2[:], p1[:], s3[:, 2 * DIM :])
                    nbs = spool.tile([P, DIM], bf16, tag="nbs")
                    nc.vector.tensor_add(
                        nbs[:], p2[:], gq[:, base + 25 * DIM : base + 26 * DIM]
                    )

                    # transpose node row and neighbour sum into [dim, item]
                    psum_t = pp.tile([P, 2 * DIM], bf16, tag="tp")
                    nc.tensor.matmul(
                        out=psum_t[:, :DIM],
                        lhsT=gq[:, base : base + DIM],
                        rhs=ident[:],
                        is_transpose=True,
                        start=True,
                        stop=True,
                    )
                    nc.tensor.matmul(
                        out=psum_t[:, DIM:],
                        lhsT=nbs[:],
                        rhs=ident[:],
                        is_transpose=True,
                        start=True,
                        stop=True,
                    )
                    catT = opool.tile([P, 2 * DIM], bf16, tag="catT")
                    nc.scalar.activation(out=catT[:], in_=psum_t[:], func=copyf)

                    psum_o = pp.tile([P, UNITS], f32, tag="o")
                    nc.tensor.matmul(
                        out=psum_o[:],
                        lhsT=catT[:, :DIM],
                        rhs=wt_sb[:],
                        start=True,
                        stop=False,
                    )
                    nc.tensor.matmul(
                        out=psum_o[:],
                        lhsT=catT[:, DIM:],
                        rhs=wb_sb[:],
                        start=False,
                        stop=True,
                    )
                    o_sb = opool.tile([P, UNITS], f32, tag="osb")
                    nc.scalar.activation(out=o_sb[:], in_=psum_o[:], func=relu)
                    row = q * Q_ITEMS + t * T_ITEMS
                    nc.sync.dma_start(
                        out=out[row : row + T_ITEMS, :], in_=o_sb[:]
                    )

    nc.compile()
    return nc


_PROGRAM_CACHE = {}


def _get_program(n_items):
    if n_items not in _PROGRAM_CACHE:
        _PROGRAM_CACHE[n_items] = build_program(n_items)
    return _PROGRAM_CACHE[n_items]


def _prep_core(features_bf, idx_core):
    """Per-core host prep: dedup per quarter, build sub-tables + wrapped
    int16 tile/slot-major indices."""
    n_q = idx_core.shape[0] // Q_ITEMS
    subtab = np.zeros((n_q, U_MAX, DIM), dtype=BF16_NP)
    idx16 = np.empty((n_q, P, IDX_COLS), dtype=np.int16)
    for q in range(n_q):
        sl = idx_core[q * Q_ITEMS : (q + 1) * Q_ITEMS]  # [Q_ITEMS, IDX_W]
        uniq, inv = np.unique(sl, return_inverse=True)
        assert len(uniq) <= U_MAX, f"unique rows {len(uniq)} > {U_MAX}"
        subtab[q, : len(uniq)] = features_bf[uniq]
        inv = inv.reshape(Q_ITEMS, IDX_W).astype(np.int16)
        # index order j = (t*IDX_W + c)*128 + p
        ordered = np.concatenate(
            [
                inv[t * T_ITEMS : (t + 1) * T_ITEMS].T.ravel()
                for t in range(N_TILES)
            ]
        )
        # wrap each gather chunk into 16 partitions (within-chunk
        # j = col*16 + p), replicate to 128
        wrapped = np.concatenate(
            [
                ordered[k * NI_CHUNK : (k + 1) * NI_CHUNK].reshape(-1, 16).T
                for k in range(N_CHUNK)
            ],
            axis=1,
        )
        idx16[q] = np.tile(wrapped, (8, 1))
    return subtab, idx16


def _prep_inputs(features, node, neighbours, W):
    features_bf = np.asarray(features, dtype=np.float32).astype(BF16_NP)
    node = np.asarray(node, dtype=np.int32).reshape(-1, 1)
    neighbours = np.asarray(neighbours, dtype=np.int32)
    W = np.asarray(W, dtype=np.float32)
    idx_all = np.ascontiguousarray(
        np.concatenate([node, neighbours], axis=1), dtype=np.int32
    )
    wt = np.ascontiguousarray(W[:DIM]).astype(BF16_NP)
    wb = (W[DIM:].astype(np.float64) / K).astype(BF16_NP)
    return features_bf, idx_all, wt, wb


def kernel(features, node, neighbours, W, trace=False):
    features_bf, idx_all, wt, wb = _prep_inputs(features, node, neighbours, W)
    n_total = idx_all.shape[0]
    per_core = n_total // N_CORES
    nc = _get_program(per_core)
    in_maps = []
    for i in range(N_CORES):
        subtab, idx16 = _prep_core(
            features_bf, idx_all[i * per_core : (i + 1) * per_core]
        )
        in_maps.append({"subtab": subtab, "idx16": idx16, "wt": wt, "wb": wb})
    res = run_bass_kernel_spmd(nc, in_maps, list(range(N_CORES)), trace=trace)
    out = np.concatenate([res.results[i]["out"] for i in range(N_CORES)], axis=0)
    if trace:
        kernel.last_result = res
    return out


# revision 19
# speedup vs baseline: 2.7442x; 1.0235x over previous
"""Trainium2 Bass kernel for nn_MeanAggregator (GNN mean aggregation).

out[b] = relu(concat(features[node[b]], mean_k features[neighbours[b,k]]) @ W)

8 NeuronCores, data-parallel over the batch (4096 items/core).  Tolerance is
2e-2, so features/W are cast to bf16 on the host (measured end-to-end error
~2.5e-3).

Gather strategy: the only SWDGE path whose Q7 descriptor generation is
vectorized is `dma_gather` (~3.5 ns/row vs ~1.1 us per 128-row
`indirect_dma_start`).  Its indices are int16 (15-bit usable), so the host
builds, per quarter-core (1024 items), a deduplicated sub-table of the
~23.4k unique referenced rows (< 32767) and remaps indices into it; the
device does the full 26-rows/item expansion from HBM.  One dma_gather is
capped at 1024 indices (256 B rows; the descriptor path tops out at 2^18
bytes — HW-probed), so each quarter issues 26 chunks of 1024 rows, spread
round-robin over 4 SWDGE queues so all four Q7 core-pairs generate
descriptors in parallel.  transpose=True (XBAR spray) is NOT used: under
multi-queue load its rx/tx rings desync and chunk k+1's data lands on chunk
k's columns (HW-probed, deterministic).  Plain gathers put row j in
partition j%128, block j//128, so ordering indices as j = (tile*26 + slot)
*128 + p reproduces the [item-partition, slot-block] layout directly.

Compute per 128-item tile: DVE pairwise tree (bf16, 2 elem/cyc) sums the 25
neighbour slots; PE transposes node + neighbour-sum into [dim, item] (2
transpose matmuls) and applies W_top / W_bot/25 with PSUM accumulation; ACT
does the PSUM->SBUF copy and the relu so DVE/PE stay on their critical
paths.
"""

import sys

sys.path.insert(0, "/opt/trn_rl_repo")

import numpy as np

from concourse import bacc, bass, mybir, tile
from concourse.bass_utils import run_bass_kernel_spmd
from concourse.masks import make_identity

N_NODES = 100000
DIM = 128
B = 32768
K = 25
UNITS = 128
N_CORES = 8
P = 128
IDX_W = K + 1

Q_ITEMS = 1024  # items per gather batch (quarter of a core)
T_ITEMS = 128  # items per compute tile
N_TILES = Q_ITEMS // T_ITEMS
NI = IDX_W * Q_ITEMS  # indices per gather batch (26624)
TI = IDX_W * T_ITEMS  # indices per tile (3328)
IDX_COLS = NI // 16
U_MAX = 25000  # sub-table capacity (unique rows per batch ~23.4k +- 0.1k)
NI_CHUNK = 1024  # max indices per dma_gather (non-transpose, HW-probed)
N_CHUNK = NI // NI_CHUNK  # 26 chunks per quarter

BF16_NP = mybir.dt.np(mybir.dt.bfloat16)


def build_program(n_items):
    n_q = n_items // Q_ITEMS
    assert n_items % Q_ITEMS == 0

    nc = bacc.Bacc("TRN2", target_bir_lowering=False, debug=False,
                   num_swdge_queues=4)
    f32 = mybir.dt.float32
    bf16 = mybir.dt.bfloat16
    subtab = nc.dram_tensor(
        "subtab", [n_q, U_MAX, DIM], bf16, kind="ExternalInput"
    ).ap()
    idx16 = nc.dram_tensor(
        "idx16", [n_q, P, IDX_COLS], mybir.dt.int16, kind="ExternalInput"
    ).ap()
    wt = nc.dram_tensor("wt", [DIM, UNITS], bf16, kind="ExternalInput").ap()
    wb = nc.dram_tensor("wb", [DIM, UNITS], bf16, kind="ExternalInput").ap()
    out = nc.dram_tensor("out", [n_items, UNITS], f32, kind="ExternalOutput").ap()

    relu = mybir.ActivationFunctionType.Relu
    copyf = mybir.ActivationFunctionType.Copy

    with tile.TileContext(nc) as tc:
        with (
            tc.tile_pool(name="const", bufs=1) as cpool,
            tc.tile_pool(name="gpool", bufs=2) as gpool,
            tc.tile_pool(name="spool", bufs=2) as spool,
            tc.tile_pool(name="opool", bufs=3) as opool,
            tc.tile_pool(name="pp", bufs=2, space="PSUM") as pp,
            tc.tile_pool(name="ppw", bufs=1, space="PSUM") as ppw,
        ):
            wt_sb = cpool.tile([DIM, UNITS], bf16, tag="wt")
            nc.sync.dma_start(out=wt_sb[:], in_=wt[:])
            wb_sb = cpool.tile([DIM, UNITS], bf16, tag="wb")
            nc.sync.dma_start(out=wb_sb[:], in_=wb[:])
            ident = cpool.tile([P, P], bf16, tag="ident")
            make_identity(nc, ident[:])

            # idx_sb[p, q*IDX_COLS + c] = idx16[q, p, c]
            idx_sb = cpool.tile([P, n_q * IDX_COLS], mybir.dt.int16, tag="idx")
            nc.sync.dma_start(
                out=idx_sb[:].rearrange("p (q c) -> p q c", c=IDX_COLS),
                in_=idx16.rearrange("q p c -> p q c"),
            )

            # warmup matmuls absorb the constant-load waits
            psum_warm_t = ppw.tile([P, UNITS], bf16, tag="warmt")
            nc.tensor.matmul(
                out=psum_warm_t[:], lhsT=ident[:], rhs=ident[:], is_transpose=True
            )
            psum_warm = ppw.tile([P, UNITS], f32, tag="warm")
            nc.tensor.matmul(out=psum_warm[:], lhsT=wt_sb[:], rhs=wt_sb[:])
            nc.tensor.matmul(out=psum_warm[:], lhsT=wb_sb[:], rhs=wb_sb[:])

            chunk_no = 0
            for q in range(n_q):
                # plain gather of 26624 rows: row j -> partition j%128,
                # block j//128; j = (t*26 + c)*128 + p, so tile t's slot c
                # sits at gq[:, (t*26+c)*128 : +128]
                gq = gpool.tile([P, NI], bf16, tag="gq")
                for k in range(N_CHUNK):
                    a = k * NI_CHUNK
                    nc.gpsimd.dma_gather(
                        gq[:, a : a + NI_CHUNK].rearrange(
                            "p (n e) -> p n e", e=DIM
                        ),
                        subtab[q],
                        idx_sb[
                            :,
                            q * IDX_COLS + a // 16 : q * IDX_COLS
                            + (a + NI_CHUNK) // 16,
                        ],
                        NI_CHUNK,
                        NI_CHUNK,
                        DIM,
                        transpose=False,
                        single_packet=False,
                        queue_num=chunk_no % 4,
                    )
                    chunk_no += 1

                for t in range(N_TILES):
                    base = t * TI  # element column offset of tile t
                    # DVE pairwise tree over neighbour slots 1..25
                    s12 = spool.tile([P, 12 * DIM], bf16, tag="s12")
                    nc.vector.tensor_add(
                        s12[:],
                        gq[:, base + 1 * DIM : base + 13 * DIM],
                        gq[:, base + 13 * DIM : base + 25 * DIM],
                    )
                    s6 = spool.tile([P, 6 * DIM], bf16, tag="s6")
                    nc.vector.tensor_add(
                        s6[:], s12[:, : 6 * DIM], s12[:, 6 * DIM :]
                    )
                    s3 = spool.tile([P, 3 * DIM], bf16, tag="s3")
                    nc.vector.tensor_add(
                        s3[:], s6[:, : 3 * DIM], s6[:, 3 * DIM :]
                    )
                    p1 = spool.tile([P, DIM], bf16, tag="p1")
                    nc.vector.tensor_add(
                        p1[:], s3[:, :DIM], s3[:, DIM : 2 * DIM]
                    )
                    p2 = spool.tile([P, DIM], bf16, tag="p2")
                    nc.vector.tensor_add(p2[:], p1[:], s3[:, 2 * DIM :])
                    nbs = spool.tile([P, DIM], bf16, tag="nbs")
                    nc.vector.tensor_add(
                        nbs[:], p2[:], gq[:, base + 25 * DIM : base + 26 * DIM]
                    )

                    # transpose node row and neighbour sum into [dim, item]
                    psum_t = pp.tile([P, 2 * DIM], bf16, tag="tp")
                    nc.tensor.matmul(
                        out=psum_t[:, :DIM],
                        lhsT=gq[:, base : base + DIM],
                        rhs=ident[:],
                        is_transpose=True,
                        start=True,
                        stop=True,
                    )
                    nc.tensor.matmul(
                        out=psum_t[:, DIM:],
                        lhsT=nbs[:],
                        rhs=ident[:],
                        is_transpose=True,
                        start=True,
                        stop=True,
                    )
                    catT = opool.tile([P, 2 * DIM], bf16, tag="catT")
                    nc.scalar.activation(out=catT[:], in_=psum_t[:], func=copyf)

                    psum_o = pp.tile([P, UNITS], f32, tag="o")
                    nc.tensor.matmul(
                        out=psum_o[:],
                        lhsT=catT[:, :DIM],
                        rhs=wt_sb[:],
                        start=True,
                        stop=False,
                    )
                    nc.tensor.matmul(
                        out=psum_o[:],
                        lhsT=catT[:, DIM:],
                        rhs=wb_sb[:],
                        start=False,
                        stop=True,
                    )
                    o_sb = opool.tile([P, UNITS], f32, tag="osb")
                    nc.scalar.activation(out=o_sb[:], in_=psum_o[:], func=relu)
                    row = q * Q_ITEMS + t * T_ITEMS
                    nc.sync.dma_start(
                        out=out[row : row + T_ITEMS, :], in_=o_sb[:]
                    )

    nc.compile()
    return nc


_PROGRAM_CACHE = {}


def _get_program(n_items):
    if n_items not in _PROGRAM_CACHE:
        _PROGRAM_CACHE[n_items] = build_program(n_items)
    return _PROGRAM_CACHE[n_items]


def _prep_core(features_bf, idx_core):
    """Per-core host prep: dedup per quarter, build sub-tables + wrapped
    int16 tile/slot-major indices."""
    n_q = idx_core.shape[0] // Q_ITEMS
    subtab = np.zeros((n_q, U_MAX, DIM), dtype=BF16_NP)
    idx16 = np.empty((n_q, P, IDX_COLS), dtype=np.int16)
    for q in range(n_q):
        sl = idx_core[q * Q_ITEMS : (q + 1) * Q_ITEMS]  # [Q_ITEMS, IDX_W]
        uniq, inv = np.unique(sl, return_inverse=True)
        assert len(uniq) <= U_MAX, f"unique rows {len(uniq)} > {U_MAX}"
        subtab[q, : len(uniq)] = features_bf[uniq]
        inv = inv.reshape(Q_ITEMS, IDX_W).astype(np.int16)
        # index order j = (t*IDX_W + c)*128 + p
        ordered = np.concatenate(
            [
                inv[t * T_ITEMS : (t + 1) * T_ITEMS].T.ravel()
                for t in range(N_TILES)
            ]
        )
        # wrap each gather chunk into 16 partitions (within-chunk
        # j = col*16 + p), replicate to 128
        wrapped = np.concatenate(
            [
                ordered[k * NI_CHUNK : (k + 1) * NI_CHUNK].reshape(-1, 16).T
                for k in range(N_CHUNK)
            ],
            axis=1,
        )
        idx16[q] = np.tile(wrapped, (8, 1))
    return subtab, idx16


def _prep_inputs(features, node, neighbours, W):
    features_bf = np.asarray(features, dtype=np.float32).astype(BF16_NP)
    node = np.asarray(node, dtype=np.int32).reshape(-1, 1)
    neighbours = np.asarray(neighbours, dtype=np.int32)
    W = np.asarray(W, dtype=np.float32)
    idx_all = np.ascontiguousarray(
        np.concatenate([node, neighbours], axis=1), dtype=np.int32
    )
    wt = np.ascontiguousarray(W[:DIM]).astype(BF16_NP)
    wb = (W[DIM:].astype(np.float64) / K).astype(BF16_NP)
    return features_bf, idx_all, wt, wb


def kernel(features, node, neighbours, W, trace=False):
    features_bf, idx_all, wt, wb = _prep_inputs(features, node, neighbours, W)
    n_total = idx_all.shape[0]
    per_core = n_total // N_CORES
    nc = _get_program(per_core)
    in_maps = []
    for i in range(N_CORES):
        subtab, idx16 = _prep_core(
            features_bf, idx_all[i * per_core : (i + 1) * per_core]
        )
        in_maps.append({"subtab": subtab, "idx16": idx16, "wt": wt, "wb": wb})
    res = run_bass_kernel_spmd(nc, in_maps, list(range(N_CORES)), trace=trace)
    out = np.concatenate([res.results[i]["out"] for i in range(N_CORES)], axis=0)
    if trace:
        kernel.last_result = res
    return out
